# revision 17
# baseline (speedup 1.0000x reference)
"""Interval-softmax diagonal bounds kernel for Trainium2 (8 NeuronCores).

Math (per row b, element i), identical to the reference after rewriting:
    e_u = exp(u), S_u = sum_j e_u[:, j]
    lower = e_l / (e_l - e_u + S_u)
    upper = e_u / (e_u - e_l + S_l)

Memory-bound problem: trade precision for bandwidth inside the 2e-2
tolerance (measured end-to-end max rel err ~0.7e-2):
  - inputs cast to fp16 on the host (|x| <= ~5.6 so abs err <= 2.8e-3,
    exp rel err <= 0.28%),
  - outputs leave the chip as bf16 (rel err <= 0.2%; fp16 would flush
    the ~1e-6 smallest outputs to subnormals),
  halving HBM traffic to 8 MiB/core (~23.4 us at 358 GB/s per core).
Each DMA piece is its own contiguous dram tensor (host packs/unpacks)
so every transfer is a sequential HBM burst -- column-sliced views of
a packed [ROWS, W] tensor were 2-4 KiB chunks at 8 KiB stride and ran
at roughly half rate (worth ~1.5 us end to end).

Compute per 128-row block:
    ScalarE: exp(l)+rowsum, exp(u)+rowsum   (~2.0 us each)
    VectorE: 2x custom fused DVE op (8/8 ALU stages, ~2.3 us each):
        out = Src0 * recip1((Src0 - Src1) + C0)
    where recip1 is the bitcast-NOT seeded reciprocal with ONE
    Newton-Raphson step (minimax consts from RECIP_APPROX_FAST_CONSTS,
    max rel err 0.173%; the 2nd NR step is dropped to fit the final
    multiply into the 8-stage pipeline). Registered into
    concourse.dve_ops.OPS at import time (the documented extension
    point; shas computed in-process).

Schedule notes (from perfetto traces): HWDGE issues ride the serial
Sync sequencer and an output-DMA's semaphore wait blocks every later
issue on that ring, so all input DMAs are emitted before any output.
Queued transfers share HBM bandwidth, so inputs are split to match the
ACT consumption order: block 0 arrives as four 256 KiB quarters
(u first -- the lower-side DVE op needs S_u) with column-half exps
chasing them, blocks 1-2 as l/u halves, block 3 whole (it arrives
during compute).  Block 3 splits exp(l) plus the DVE ops and stores
into column halves so the serial tail after the last ACTIVATE is
short.  Measured 38.7-39.4 us on HW (baseline 69.8): ~7.2 us fixed
NEFF preamble, ~7.2 us first-data + pre-DVE exp head, ~19.6 us
gap-free DVE stream, ~4.8 us store receipt + final barrier.
"""

import os
import sys

import numpy as np

_REPO = "/opt/trn_rl_repo"
if _REPO not in sys.path:
    sys.path.insert(0, _REPO)

B, N = 4096, 2048
N_CORES = 8
ROWS = B // N_CORES  # 512 rows per core
P = 128
NBLK = ROWS // P     # 4 row-blocks per core
W = 2 * N            # packed l|u (and lower|upper) width
H = N // 2           # column half

_OP_NAME = "INTERVAL_SM_RECIP_MUL_ANT"
_SEED_C = -0.23549792   # Chebyshev seed scale (C1)
_NR_C = 2.0017324       # minimax 1-NR constant (C2)

_cache = {}


def _register_dve_op():
    """out = Src0 * recip1((Src0 - Src1) + C0); C0 = per-partition row sum.

    recip1: nx = bitnot(x); y0 = nx*C1; r = y0*(C2 - x*y0). 8 ALU
    stages exactly.
    """
    import concourse.dve_ops as dve_ops
    from concourse.dve_spec import (
        AluOp,
        Bin,
        C0,
        C1,
        C2,
        Spec,
        Src0,
        Src1,
        _has_src1,
        lower,
    )
    from concourse.dve_uop import DveOpSpec

    for o in dve_ops.OPS:
        if o.name == _OP_NAME:
            return o

    x = (Src0 - Src1) + C0
    nx = Bin(AluOp.BITWISE_NOT, x, x)
    y0 = nx * C1
    y1 = y0 * (C2 - x * y0)
    body = y1 * Src0

    def _ref(in0, in1, s0, s1, imm2):
        xx = (in0.astype(np.float32) - in1 + s0).astype(np.float32)
        nxx = (~xx.view(np.int32)).view(np.float32)
        yy0 = (nxx * np.float32(s1)).astype(np.float32)
        yy1 = (yy0 * (np.float32(imm2) - xx * yy0)).astype(np.float32)
        return (yy1 * in0).astype(np.float32)

    spec = Spec(body=body, reference=_ref)
    row = dve_ops._CUSTOM_DVE_ROW_BASE + len(dve_ops.OPS)
    assert row < 0x20, "custom-DVE opcode rows exhausted"
    shas = {}
    for ver in ("v3", "v4"):
        s = DveOpSpec(
            name=_OP_NAME,
            opcode=row,
            uops=lower(spec, ver=ver),
            rd1_en=_has_src1(spec),
        )
        shas[ver] = s.sha(ver)
    op = dve_ops.DveOp(_OP_NAME, spec, subdim=False, uops_sha=shas)
    dve_ops.OPS.append(op)
    dve_ops._SUB_OPCODE_FOR_NAME[_OP_NAME] = row
    dve_ops.CUSTOM_DVE_SPECS[_OP_NAME] = spec
    return op


def _build():
    import concourse.bacc as bacc
    import concourse.mybir as mybir
    import concourse.tile as tile

    op = _register_dve_op()
    f16 = mybir.dt.float16
    bf16 = mybir.dt.bfloat16
    f32 = mybir.dt.float32
    Exp = mybir.ActivationFunctionType.Exp
    Add = mybir.AluOpType.add
    nc = bacc.Bacc(
        "TRN2", target_bir_lowering=False, debug=False, num_devices=N_CORES
    )

    # Every DMA piece gets its own contiguous dram tensor: column-sliced
    # views of a [ROWS, W] tensor are 2-4 KiB chunks at 8 KiB stride in
    # HBM (row-buffer thrash, ~half rate); per-piece tensors make every
    # transfer a fully sequential HBM burst. The host packs/unpacks.
    i_b0 = [
        nc.dram_tensor(f"b0_{t}", [P, H], f16, kind="ExternalInput")
        for t in ("uh0", "uh1", "lh0", "lh1")
    ]
    i_b1 = [
        nc.dram_tensor(f"b1_{t}", [P, N], f16, kind="ExternalInput")
        for t in ("l", "u")
    ]
    i_b2 = [
        nc.dram_tensor(f"b2_{t}", [P, N], f16, kind="ExternalInput")
        for t in ("l", "u")
    ]
    i_b3 = nc.dram_tensor("b3_xu", [P, W], f16, kind="ExternalInput")
    o_b0 = [
        nc.dram_tensor(f"o0_{t}", [P, N], bf16, kind="ExternalOutput")
        for t in ("lo", "up")
    ]
    o_b1 = nc.dram_tensor("o1", [P, W], bf16, kind="ExternalOutput")
    o_b2 = nc.dram_tensor("o2", [P, W], bf16, kind="ExternalOutput")
    o_b3 = [
        nc.dram_tensor(f"o3_{t}", [P, H], bf16, kind="ExternalOutput")
        for t in ("loh0", "loh1", "uph0", "uph1")
    ]

    with tile.TileContext(nc) as tc:
        with (
            tc.tile_pool(name="io", bufs=3) as io,
            tc.tile_pool(name="eb", bufs=3) as eb,
            tc.tile_pool(name="ob", bufs=3) as ob,
            tc.tile_pool(name="stats", bufs=8) as st,
        ):
            # Phase 1: all input DMAs up front (io bufs cover all 4
            # blocks) so no output-DMA wait ever stalls an input issue.
            # Block 0 streams in as quarters, u first, so the first
            # lower-side DVE op can start as early as possible.
            xus = []
            for b in range(NBLK):
                xu = io.tile([P, W], f16, tag="xu")
                if b == 0:
                    nc.sync.dma_start(out=xu[:, N : N + H], in_=i_b0[0][:, :])
                    nc.sync.dma_start(out=xu[:, N + H :], in_=i_b0[1][:, :])
                    nc.sync.dma_start(out=xu[:, 0:H], in_=i_b0[2][:, :])
                    nc.sync.dma_start(out=xu[:, H:N], in_=i_b0[3][:, :])
                elif b in (1, 2):
                    # halves so each block's exp_l isn't starved
                    src_lu = i_b1 if b == 1 else i_b2
                    nc.sync.dma_start(out=xu[:, :N], in_=src_lu[0][:, :])
                    nc.sync.dma_start(out=xu[:, N:], in_=src_lu[1][:, :])
                else:
                    nc.sync.dma_start(out=xu, in_=i_b3[:, :])
                xus.append(xu)

            # Phase 2: per-block compute + store.
            for b in range(NBLK):
                rows = slice(b * P, (b + 1) * P)
                xu = xus[b]
                e = eb.tile([P, W], f32, tag="e")
                s = st.tile([P, 6], f32, tag="s")
                o = ob.tile([P, W], bf16, tag="o")

                if b == 0:
                    # cols: s0=S_l_h0, s1=S_l_h1, s2=S_u_h0, s3=S_u_h1,
                    #       s4=S_u, s5=S_l
                    nc.scalar.activation(
                        e[:, N : N + H], xu[:, N : N + H], Exp,
                        accum_out=s[:, 2:3],
                    )
                    nc.scalar.activation(
                        e[:, N + H :], xu[:, N + H :], Exp, accum_out=s[:, 3:4]
                    )
                    nc.scalar.activation(
                        e[:, 0:H], xu[:, 0:H], Exp, accum_out=s[:, 0:1]
                    )
                    nc.scalar.activation(
                        e[:, H:N], xu[:, H:N], Exp, accum_out=s[:, 1:2]
                    )
                    nc.vector.tensor_scalar(
                        s[:, 4:5], s[:, 2:3], s[:, 3:4], None, op0=Add
                    )
                    nc.vector._custom_dve(
                        op, out=o[:, 0:H], in0=e[:, 0:H], in1=e[:, N : N + H],
                        s0=s[:, 4:5], s1=_SEED_C, imm2=_NR_C,
                    )
                    nc.vector._custom_dve(
                        op, out=o[:, H:N], in0=e[:, H:N], in1=e[:, N + H :],
                        s0=s[:, 4:5], s1=_SEED_C, imm2=_NR_C,
                    )
                    nc.sync.dma_start(out=o_b0[0][:, :], in_=o[:, :N])
                    nc.vector.tensor_scalar(
                        s[:, 5:6], s[:, 0:1], s[:, 1:2], None, op0=Add
                    )
                    nc.vector._custom_dve(
                        op, out=o[:, N:], in0=e[:, N:], in1=e[:, :N],
                        s0=s[:, 5:6], s1=_SEED_C, imm2=_NR_C,
                    )
                    nc.sync.dma_start(out=o_b0[1][:, :], in_=o[:, N:])
                elif b < NBLK - 1:
                    # cols: s[:,0]=S_l, s[:,1]=S_u
                    nc.scalar.activation(
                        e[:, :N], xu[:, :N], Exp, accum_out=s[:, 0:1]
                    )
                    nc.scalar.activation(
                        e[:, N:], xu[:, N:], Exp, accum_out=s[:, 1:2]
                    )
                    # lower = e_l * recip1(e_l - e_u + S_u)
                    nc.vector._custom_dve(
                        op, out=o[:, :N], in0=e[:, :N], in1=e[:, N:],
                        s0=s[:, 1:2], s1=_SEED_C, imm2=_NR_C,
                    )
                    # upper = e_u * recip1(e_u - e_l + S_l)
                    nc.vector._custom_dve(
                        op, out=o[:, N:], in0=e[:, N:], in1=e[:, :N],
                        s0=s[:, 0:1], s1=_SEED_C, imm2=_NR_C,
                    )
                    nc.sync.dma_start(
                        out=(o_b1 if b == 1 else o_b2)[:, :], in_=o
                    )
                else:
                    # Last block: exp(u) first, then exp(l) in column
                    # halves; the lower-side DVE ops chase the halves,
                    # and upper (gated on full S_l) runs in halves with
                    # quarter stores so the post-ACT tail is short.
                    # cols: s[:,0]=S_l_h0, s[:,1]=S_l_h1, s[:,2]=S_u,
                    #       s[:,3]=S_l
                    nc.scalar.activation(
                        e[:, N:], xu[:, N:], Exp, accum_out=s[:, 2:3]
                    )
                    nc.scalar.activation(
                        e[:, 0:H], xu[:, 0:H], Exp, accum_out=s[:, 0:1]
                    )
                    nc.scalar.activation(
                        e[:, H:N], xu[:, H:N], Exp, accum_out=s[:, 1:2]
                    )
                    nc.vector._custom_dve(
                        op, out=o[:, 0:H], in0=e[:, 0:H], in1=e[:, N : N + H],
                        s0=s[:, 2:3], s1=_SEED_C, imm2=_NR_C,
                    )
                    nc.sync.dma_start(out=o_b3[0][:, :], in_=o[:, 0:H])
                    nc.vector._custom_dve(
                        op, out=o[:, H:N], in0=e[:, H:N], in1=e[:, N + H :],
                        s0=s[:, 2:3], s1=_SEED_C, imm2=_NR_C,
                    )
                    nc.sync.dma_start(out=o_b3[1][:, :], in_=o[:, H:N])
                    nc.vector.tensor_scalar(
                        s[:, 3:4], s[:, 0:1], s[:, 1:2], None, op0=Add
                    )
                    nc.vector._custom_dve(
                        op, out=o[:, N : N + H], in0=e[:, N : N + H],
                        in1=e[:, 0:H], s0=s[:, 3:4], s1=_SEED_C, imm2=_NR_C,
                    )
                    nc.sync.dma_start(
                        out=o_b3[2][:, :], in_=o[:, N : N + H]
                    )
                    nc.vector._custom_dve(
                        op, out=o[:, N + H :], in0=e[:, N + H :],
                        in1=e[:, H:N], s0=s[:, 3:4], s1=_SEED_C, imm2=_NR_C,
                    )
                    nc.sync.dma_start(
                        out=o_b3[3][:, :], in_=o[:, N + H :]
                    )

    nc.compile()
    return nc


def _get_nc():
    if "nc" not in _cache:
        _cache["nc"] = _build()
    return _cache["nc"]


def kernel(l: np.ndarray, u: np.ndarray):
    from concourse import bass_utils

    assert l.shape == (B, N) and u.shape == (B, N)
    lh = np.ascontiguousarray(l, dtype=np.float16)
    uh = np.ascontiguousarray(u, dtype=np.float16)

    def core_inputs(i):
        r = i * ROWS
        cp = np.ascontiguousarray
        return {
            "b0_uh0": cp(uh[r : r + P, 0:H]),
            "b0_uh1": cp(uh[r : r + P, H:N]),
            "b0_lh0": cp(lh[r : r + P, 0:H]),
            "b0_lh1": cp(lh[r : r + P, H:N]),
            "b1_l": cp(lh[r + P : r + 2 * P]),
            "b1_u": cp(uh[r + P : r + 2 * P]),
            "b2_l": cp(lh[r + 2 * P : r + 3 * P]),
            "b2_u": cp(uh[r + 2 * P : r + 3 * P]),
            "b3_xu": np.concatenate(
                [lh[r + 3 * P : r + 4 * P], uh[r + 3 * P : r + 4 * P]], axis=1
            ),
        }

    nc = _get_nc()
    in_maps = [core_inputs(i) for i in range(N_CORES)]
    trace = bool(int(os.environ.get("KERNEL_TRACE", "0")))
    res = bass_utils.run_bass_kernel_spmd(
        nc,
        in_maps,
        core_ids=list(range(N_CORES)),
        trace=trace,
        trace_cores=[0] if trace else None,
    )
    _cache["last_run"] = res
    lower = np.empty((B, N), dtype=np.float32)
    upper = np.empty((B, N), dtype=np.float32)
    for i, r_ in enumerate(res.results):
        r = i * ROWS
        g = lambda name: np.asarray(r_[name]).astype(np.float32)
        lower[r : r + P] = g("o0_lo")
        upper[r : r + P] = g("o0_up")
        o1 = g("o1")
        lower[r + P : r + 2 * P] = o1[:, :N]
        upper[r + P : r + 2 * P] = o1[:, N:]
        o2 = g("o2")
        lower[r + 2 * P : r + 3 * P] = o2[:, :N]
        upper[r + 2 * P : r + 3 * P] = o2[:, N:]
        lower[r + 3 * P : r + 4 * P, 0:H] = g("o3_loh0")
        lower[r + 3 * P : r + 4 * P, H:N] = g("o3_loh1")
        upper[r + 3 * P : r + 4 * P, 0:H] = g("o3_uph0")
        upper[r + 3 * P : r + 4 * P, H:N] = g("o3_uph1")
    return lower, upper


# revision 18
# speedup vs baseline: 1.0096x; 1.0096x over previous
"""Interval-softmax diagonal bounds kernel for Trainium2 (8 NeuronCores).

Math (per row b, element i), identical to the reference after rewriting:
    e_u = exp(u), S_u = sum_j e_u[:, j]
    lower = e_l / (e_l - e_u + S_u)
    upper = e_u / (e_u - e_l + S_l)

Memory-bound problem: trade precision for bandwidth inside the 2e-2
tolerance (measured end-to-end max rel err ~0.7e-2):
  - inputs cast to fp16 on the host (|x| <= ~5.6 so abs err <= 2.8e-3,
    exp rel err <= 0.28%),
  - outputs leave the chip as bf16 (rel err <= 0.2%; fp16 would flush
    the ~1e-6 smallest outputs to subnormals),
  halving HBM traffic to 8 MiB/core (~23.4 us at 358 GB/s per core).
Each DMA piece is its own contiguous dram tensor (host packs/unpacks)
so every transfer is a sequential HBM burst -- column-sliced views of
a packed [ROWS, W] tensor were 2-4 KiB chunks at 8 KiB stride and ran
at roughly half rate (worth ~1.5 us end to end).

Compute per 128-row block:
    ScalarE: exp(l)+rowsum, exp(u)+rowsum   (~2.0 us each)
    VectorE: 2x custom fused DVE op (8/8 ALU stages, ~2.3 us each):
        out = Src0 * recip1((Src0 - Src1) + C0)
    where recip1 is the bitcast-NOT seeded reciprocal with ONE
    Newton-Raphson step (minimax consts from RECIP_APPROX_FAST_CONSTS,
    max rel err 0.173%; the 2nd NR step is dropped to fit the final
    multiply into the 8-stage pipeline). Registered into
    concourse.dve_ops.OPS at import time (the documented extension
    point; shas computed in-process).

Schedule notes (from perfetto traces): HWDGE issues ride the serial
Sync sequencer and an output-DMA's semaphore wait blocks every later
issue on that ring, so all input DMAs are emitted before any output.
Queued transfers share HBM bandwidth, so inputs are split to match the
ACT consumption order: block 0 arrives as four 256 KiB quarters
(u first -- the lower-side DVE op needs S_u) with column-half exps
chasing them, blocks 1-2 as l/u halves, block 3 whole (it arrives
during compute).  Block 3 splits exp(l) plus the DVE ops and stores
into column halves so the serial tail after the last ACTIVATE is
short.  Measured 38.7-39.4 us on HW (baseline 69.8): ~7.2 us fixed
NEFF preamble, ~7.2 us first-data + pre-DVE exp head, ~19.6 us
gap-free DVE stream, ~4.8 us store receipt + final barrier.
"""

import os
import sys

import numpy as np

_REPO = "/opt/trn_rl_repo"
if _REPO not in sys.path:
    sys.path.insert(0, _REPO)

B, N = 4096, 2048
N_CORES = 8
ROWS = B // N_CORES  # 512 rows per core
P = 128
NBLK = ROWS // P     # 4 row-blocks per core
W = 2 * N            # packed l|u (and lower|upper) width
H = N // 2           # column half

_OP_NAME = "INTERVAL_SM_RECIP_MUL_ANT"
_SEED_C = -0.23549792   # Chebyshev seed scale (C1)
_NR_C = 2.0017324       # minimax 1-NR constant (C2)

_cache = {}


def _register_dve_op():
    """out = Src0 * recip1((Src0 - Src1) + C0); C0 = per-partition row sum.

    recip1: nx = bitnot(x); y0 = nx*C1; r = y0*(C2 - x*y0). 8 ALU
    stages exactly.
    """
    import concourse.dve_ops as dve_ops
    from concourse.dve_spec import (
        AluOp,
        Bin,
        C0,
        C1,
        C2,
        Spec,
        Src0,
        Src1,
        _has_src1,
        lower,
    )
    from concourse.dve_uop import DveOpSpec

    for o in dve_ops.OPS:
        if o.name == _OP_NAME:
            return o

    x = (Src0 - Src1) + C0
    nx = Bin(AluOp.BITWISE_NOT, x, x)
    y0 = nx * C1
    y1 = y0 * (C2 - x * y0)
    body = y1 * Src0

    def _ref(in0, in1, s0, s1, imm2):
        xx = (in0.astype(np.float32) - in1 + s0).astype(np.float32)
        nxx = (~xx.view(np.int32)).view(np.float32)
        yy0 = (nxx * np.float32(s1)).astype(np.float32)
        yy1 = (yy0 * (np.float32(imm2) - xx * yy0)).astype(np.float32)
        return (yy1 * in0).astype(np.float32)

    spec = Spec(body=body, reference=_ref)
    row = dve_ops._CUSTOM_DVE_ROW_BASE + len(dve_ops.OPS)
    assert row < 0x20, "custom-DVE opcode rows exhausted"
    shas = {}
    for ver in ("v3", "v4"):
        s = DveOpSpec(
            name=_OP_NAME,
            opcode=row,
            uops=lower(spec, ver=ver),
            rd1_en=_has_src1(spec),
        )
        shas[ver] = s.sha(ver)
    op = dve_ops.DveOp(_OP_NAME, spec, subdim=False, uops_sha=shas)
    dve_ops.OPS.append(op)
    dve_ops._SUB_OPCODE_FOR_NAME[_OP_NAME] = row
    dve_ops.CUSTOM_DVE_SPECS[_OP_NAME] = spec
    return op


def _build():
    import concourse.bacc as bacc
    import concourse.mybir as mybir
    import concourse.tile as tile

    op = _register_dve_op()
    f16 = mybir.dt.float16
    bf16 = mybir.dt.bfloat16
    f32 = mybir.dt.float32
    Exp = mybir.ActivationFunctionType.Exp
    Add = mybir.AluOpType.add
    nc = bacc.Bacc(
        "TRN2", target_bir_lowering=False, debug=False, num_devices=N_CORES
    )

    # Every DMA piece gets its own contiguous dram tensor: column-sliced
    # views of a [ROWS, W] tensor are 2-4 KiB chunks at 8 KiB stride in
    # HBM (row-buffer thrash, ~half rate); per-piece tensors make every
    # transfer a fully sequential HBM burst. The host packs/unpacks.
    i_b0 = [
        nc.dram_tensor(f"b0_{t}", [P, H], f16, kind="ExternalInput")
        for t in ("uh0", "uh1", "lh0", "lh1")
    ]
    i_b1 = [
        nc.dram_tensor(f"b1_{t}", [P, N], f16, kind="ExternalInput")
        for t in ("l", "u")
    ]
    i_b2 = [
        nc.dram_tensor(f"b2_{t}", [P, N], f16, kind="ExternalInput")
        for t in ("l", "u")
    ]
    i_b3 = nc.dram_tensor("b3_xu", [P, W], f16, kind="ExternalInput")
    o_b0 = [
        nc.dram_tensor(f"o0_{t}", [P, N], bf16, kind="ExternalOutput")
        for t in ("lo", "up")
    ]
    o_b1 = nc.dram_tensor("o1", [P, W], bf16, kind="ExternalOutput")
    o_b2 = nc.dram_tensor("o2", [P, W], bf16, kind="ExternalOutput")
    o_b3 = [
        nc.dram_tensor(f"o3_{t}", [P, H], bf16, kind="ExternalOutput")
        for t in ("loh0", "loh1", "uph0", "uph1")
    ]

    with tile.TileContext(nc) as tc:
        with (
            tc.tile_pool(name="io", bufs=3) as io,
            tc.tile_pool(name="eb", bufs=3) as eb,
            tc.tile_pool(name="ob", bufs=3) as ob,
            tc.tile_pool(name="stats", bufs=8) as st,
        ):
            # Phase 1: all input DMAs up front (io bufs cover all 4
            # blocks) so no output-DMA wait ever stalls an input issue.
            # Block 0 streams in as quarters, u first, so the first
            # lower-side DVE op can start as early as possible.
            xus = []
            for b in range(NBLK):
                xu = io.tile([P, W], f16, tag="xu")
                if b == 0:
                    # piece 1 rides the Scalar HWDGE ring: that sequencer
                    # clears its preamble ~0.7us before Sync, and a single
                    # early piece leaves no cross-ring contention window
                    nc.scalar.dma_start(out=xu[:, N : N + H], in_=i_b0[0][:, :])
                    nc.sync.dma_start(out=xu[:, N + H :], in_=i_b0[1][:, :])
                    nc.sync.dma_start(out=xu[:, 0:H], in_=i_b0[2][:, :])
                    nc.sync.dma_start(out=xu[:, H:N], in_=i_b0[3][:, :])
                elif b in (1, 2):
                    # halves so each block's exp_l isn't starved
                    src_lu = i_b1 if b == 1 else i_b2
                    nc.sync.dma_start(out=xu[:, :N], in_=src_lu[0][:, :])
                    nc.sync.dma_start(out=xu[:, N:], in_=src_lu[1][:, :])
                else:
                    nc.sync.dma_start(out=xu, in_=i_b3[:, :])
                xus.append(xu)

            # Phase 2: per-block compute + store.
            for b in range(NBLK):
                rows = slice(b * P, (b + 1) * P)
                xu = xus[b]
                e = eb.tile([P, W], f32, tag="e")
                s = st.tile([P, 6], f32, tag="s")
                o = ob.tile([P, W], bf16, tag="o")

                if b == 0:
                    # cols: s0=S_l_h0, s1=S_l_h1, s2=S_u_h0, s3=S_u_h1,
                    #       s4=S_u, s5=S_l
                    nc.scalar.activation(
                        e[:, N : N + H], xu[:, N : N + H], Exp,
                        accum_out=s[:, 2:3],
                    )
                    nc.scalar.activation(
                        e[:, N + H :], xu[:, N + H :], Exp, accum_out=s[:, 3:4]
                    )
                    nc.scalar.activation(
                        e[:, 0:H], xu[:, 0:H], Exp, accum_out=s[:, 0:1]
                    )
                    nc.scalar.activation(
                        e[:, H:N], xu[:, H:N], Exp, accum_out=s[:, 1:2]
                    )
                    nc.vector.tensor_scalar(
                        s[:, 4:5], s[:, 2:3], s[:, 3:4], None, op0=Add
                    )
                    nc.vector._custom_dve(
                        op, out=o[:, 0:H], in0=e[:, 0:H], in1=e[:, N : N + H],
                        s0=s[:, 4:5], s1=_SEED_C, imm2=_NR_C,
                    )
                    nc.vector._custom_dve(
                        op, out=o[:, H:N], in0=e[:, H:N], in1=e[:, N + H :],
                        s0=s[:, 4:5], s1=_SEED_C, imm2=_NR_C,
                    )
                    nc.sync.dma_start(out=o_b0[0][:, :], in_=o[:, :N])
                    nc.vector.tensor_scalar(
                        s[:, 5:6], s[:, 0:1], s[:, 1:2], None, op0=Add
                    )
                    nc.vector._custom_dve(
                        op, out=o[:, N:], in0=e[:, N:], in1=e[:, :N],
                        s0=s[:, 5:6], s1=_SEED_C, imm2=_NR_C,
                    )
                    nc.sync.dma_start(out=o_b0[1][:, :], in_=o[:, N:])
                elif b < NBLK - 1:
                    # cols: s[:,0]=S_l, s[:,1]=S_u
                    nc.scalar.activation(
                        e[:, :N], xu[:, :N], Exp, accum_out=s[:, 0:1]
                    )
                    nc.scalar.activation(
                        e[:, N:], xu[:, N:], Exp, accum_out=s[:, 1:2]
                    )
                    # lower = e_l * recip1(e_l - e_u + S_u)
                    nc.vector._custom_dve(
                        op, out=o[:, :N], in0=e[:, :N], in1=e[:, N:],
                        s0=s[:, 1:2], s1=_SEED_C, imm2=_NR_C,
                    )
                    # upper = e_u * recip1(e_u - e_l + S_l)
                    nc.vector._custom_dve(
                        op, out=o[:, N:], in0=e[:, N:], in1=e[:, :N],
                        s0=s[:, 0:1], s1=_SEED_C, imm2=_NR_C,
                    )
                    nc.sync.dma_start(
                        out=(o_b1 if b == 1 else o_b2)[:, :], in_=o
                    )
                else:
                    # Last block: exp(u) first, then exp(l) in column
                    # halves; the lower-side DVE ops chase the halves,
                    # and upper (gated on full S_l) runs in halves with
                    # quarter stores so the post-ACT tail is short.
                    # cols: s[:,0]=S_l_h0, s[:,1]=S_l_h1, s[:,2]=S_u,
                    #       s[:,3]=S_l
                    nc.scalar.activation(
                        e[:, N:], xu[:, N:], Exp, accum_out=s[:, 2:3]
                    )
                    nc.scalar.activation(
                        e[:, 0:H], xu[:, 0:H], Exp, accum_out=s[:, 0:1]
                    )
                    nc.scalar.activation(
                        e[:, H:N], xu[:, H:N], Exp, accum_out=s[:, 1:2]
                    )
                    nc.vector._custom_dve(
                        op, out=o[:, 0:H], in0=e[:, 0:H], in1=e[:, N : N + H],
                        s0=s[:, 2:3], s1=_SEED_C, imm2=_NR_C,
                    )
                    nc.sync.dma_start(out=o_b3[0][:, :], in_=o[:, 0:H])
                    nc.vector._custom_dve(
                        op, out=o[:, H:N], in0=e[:, H:N], in1=e[:, N + H :],
                        s0=s[:, 2:3], s1=_SEED_C, imm2=_NR_C,
                    )
                    nc.sync.dma_start(out=o_b3[1][:, :], in_=o[:, H:N])
                    nc.vector.tensor_scalar(
                        s[:, 3:4], s[:, 0:1], s[:, 1:2], None, op0=Add
                    )
                    nc.vector._custom_dve(
                        op, out=o[:, N : N + H], in0=e[:, N : N + H],
                        in1=e[:, 0:H], s0=s[:, 3:4], s1=_SEED_C, imm2=_NR_C,
                    )
                    nc.sync.dma_start(
                        out=o_b3[2][:, :], in_=o[:, N : N + H]
                    )
                    nc.vector._custom_dve(
                        op, out=o[:, N + H :], in0=e[:, N + H :],
                        in1=e[:, H:N], s0=s[:, 3:4], s1=_SEED_C, imm2=_NR_C,
                    )
                    nc.sync.dma_start(
                        out=o_b3[3][:, :], in_=o[:, N + H :]
                    )

    nc.compile()
    return nc


def _get_nc():
    if "nc" not in _cache:
        _cache["nc"] = _build()
    return _cache["nc"]


def kernel(l: np.ndarray, u: np.ndarray):
    from concourse import bass_utils

    assert l.shape == (B, N) and u.shape == (B, N)
    lh = np.ascontiguousarray(l, dtype=np.float16)
    uh = np.ascontiguousarray(u, dtype=np.float16)

    def core_inputs(i):
        r = i * ROWS
        cp = np.ascontiguousarray
        return {
            "b0_uh0": cp(uh[r : r + P, 0:H]),
            "b0_uh1": cp(uh[r : r + P, H:N]),
            "b0_lh0": cp(lh[r : r + P, 0:H]),
            "b0_lh1": cp(lh[r : r + P, H:N]),
            "b1_l": cp(lh[r + P : r + 2 * P]),
            "b1_u": cp(uh[r + P : r + 2 * P]),
            "b2_l": cp(lh[r + 2 * P : r + 3 * P]),
            "b2_u": cp(uh[r + 2 * P : r + 3 * P]),
            "b3_xu": np.concatenate(
                [lh[r + 3 * P : r + 4 * P], uh[r + 3 * P : r + 4 * P]], axis=1
            ),
        }

    nc = _get_nc()
    in_maps = [core_inputs(i) for i in range(N_CORES)]
    trace = bool(int(os.environ.get("KERNEL_TRACE", "0")))
    res = bass_utils.run_bass_kernel_spmd(
        nc,
        in_maps,
        core_ids=list(range(N_CORES)),
        trace=trace,
        trace_cores=[0] if trace else None,
    )
    _cache["last_run"] = res
    lower = np.empty((B, N), dtype=np.float32)
    upper = np.empty((B, N), dtype=np.float32)
    for i, r_ in enumerate(res.results):
        r = i * ROWS
        g = lambda name: np.asarray(r_[name]).astype(np.float32)
        lower[r : r + P] = g("o0_lo")
        upper[r : r + P] = g("o0_up")
        o1 = g("o1")
        lower[r + P : r + 2 * P] = o1[:, :N]
        upper[r + P : r + 2 * P] = o1[:, N:]
        o2 = g("o2")
        lower[r + 2 * P : r + 3 * P] = o2[:, :N]
        upper[r + 2 * P : r + 3 * P] = o2[:, N:]
        lower[r + 3 * P : r + 4 * P, 0:H] = g("o3_loh0")
        lower[r + 3 * P : r + 4 * P, H:N] = g("o3_loh1")
        upper[r + 3 * P : r + 4 * P, 0:H] = g("o3_uph0")
        upper[r + 3 * P : r + 4 * P, H:N] = g("o3_uph1")
    return lower, upper


# revision 19
# speedup vs baseline: 1.0374x; 1.0275x over previous
"""Interval-softmax diagonal bounds kernel for Trainium2 (8 NeuronCores).

Math (per row b, element i), identical to the reference after rewriting:
    e_u = exp(u), S_u = sum_j e_u[:, j]
    lower = e_l / (e_l - e_u + S_u)
    upper = e_u / (e_u - e_l + S_l)

Memory-bound problem: trade precision for bandwidth inside the 2e-2
tolerance (measured end-to-end max rel err ~0.7e-2):
  - inputs cast to fp16 on the host (|x| <= ~5.6 so abs err <= 2.8e-3,
    exp rel err <= 0.28%),
  - outputs leave the chip as bf16 (rel err <= 0.2%; fp16 would flush
    the ~1e-6 smallest outputs to subnormals),
  halving HBM traffic to 8 MiB/core (~23.4 us at 358 GB/s per core).
Each DMA piece is its own contiguous dram tensor (host packs/unpacks)
so every transfer is a sequential HBM burst -- column-sliced views of
a packed [ROWS, W] tensor were 2-4 KiB chunks at 8 KiB stride and ran
at roughly half rate (worth ~1.5 us end to end).

Compute per 128-row block:
    ScalarE: exp(l)+rowsum, exp(u)+rowsum   (~2.0 us each)
    VectorE: 2x custom fused DVE op (8/8 ALU stages, ~2.3 us each):
        out = Src0 * recip1((Src0 - Src1) + C0)
    where recip1 is the bitcast-NOT seeded reciprocal with ONE
    Newton-Raphson step (minimax consts from RECIP_APPROX_FAST_CONSTS,
    max rel err 0.173%; the 2nd NR step is dropped to fit the final
    multiply into the 8-stage pipeline). Registered into
    concourse.dve_ops.OPS at import time (the documented extension
    point; shas computed in-process).

Schedule notes (from perfetto traces): HWDGE issues ride the serial
Sync sequencer and an output-DMA's semaphore wait blocks every later
issue on that ring, so all input DMAs are emitted before any output.
Queued transfers share HBM bandwidth, so inputs are split to match the
ACT consumption order: block 0 arrives as four 256 KiB quarters
(u first -- the lower-side DVE op needs S_u) with column-half exps
chasing them, blocks 1-2 as l/u halves, block 3 whole (it arrives
during compute).  Block 3 splits exp(l) plus the DVE ops and stores
into column halves so the serial tail after the last ACTIVATE is
short.  Measured 38.7-39.4 us on HW (baseline 69.8): ~7.2 us fixed
NEFF preamble, ~7.2 us first-data + pre-DVE exp head, ~19.6 us
gap-free DVE stream, ~4.8 us store receipt + final barrier.
"""

import os
import sys

import numpy as np

_REPO = "/opt/trn_rl_repo"
if _REPO not in sys.path:
    sys.path.insert(0, _REPO)

B, N = 4096, 2048
N_CORES = 8
ROWS = B // N_CORES  # 512 rows per core
P = 128
NBLK = ROWS // P     # 4 row-blocks per core
W = 2 * N            # packed l|u (and lower|upper) width
H = N // 2           # column half

_OP_NAME = "INTERVAL_SM_RECIP_MUL_ANT"
_SEED_C = -0.23549792   # Chebyshev seed scale (C1)
_NR_C = 2.0017324       # minimax 1-NR constant (C2)

_cache = {}


def _register_dve_op():
    """out = Src0 * recip1((Src0 - Src1) + C0); C0 = per-partition row sum.

    recip1: nx = bitnot(x); y0 = nx*C1; r = y0*(C2 - x*y0). 8 ALU
    stages exactly.
    """
    import concourse.dve_ops as dve_ops
    from concourse.dve_spec import (
        AluOp,
        Bin,
        C0,
        C1,
        C2,
        Spec,
        Src0,
        Src1,
        _has_src1,
        lower,
    )
    from concourse.dve_uop import DveOpSpec

    for o in dve_ops.OPS:
        if o.name == _OP_NAME:
            return o

    x = (Src0 - Src1) + C0
    nx = Bin(AluOp.BITWISE_NOT, x, x)
    y0 = nx * C1
    y1 = y0 * (C2 - x * y0)
    body = y1 * Src0

    def _ref(in0, in1, s0, s1, imm2):
        xx = (in0.astype(np.float32) - in1 + s0).astype(np.float32)
        nxx = (~xx.view(np.int32)).view(np.float32)
        yy0 = (nxx * np.float32(s1)).astype(np.float32)
        yy1 = (yy0 * (np.float32(imm2) - xx * yy0)).astype(np.float32)
        return (yy1 * in0).astype(np.float32)

    spec = Spec(body=body, reference=_ref)
    row = dve_ops._CUSTOM_DVE_ROW_BASE + len(dve_ops.OPS)
    assert row < 0x20, "custom-DVE opcode rows exhausted"
    shas = {}
    for ver in ("v3", "v4"):
        s = DveOpSpec(
            name=_OP_NAME,
            opcode=row,
            uops=lower(spec, ver=ver),
            rd1_en=_has_src1(spec),
        )
        shas[ver] = s.sha(ver)
    op = dve_ops.DveOp(_OP_NAME, spec, subdim=False, uops_sha=shas)
    dve_ops.OPS.append(op)
    dve_ops._SUB_OPCODE_FOR_NAME[_OP_NAME] = row
    dve_ops.CUSTOM_DVE_SPECS[_OP_NAME] = spec
    return op


def _build():
    import concourse.bacc as bacc
    import concourse.mybir as mybir
    import concourse.tile as tile

    op = _register_dve_op()
    f16 = mybir.dt.float16
    bf16 = mybir.dt.bfloat16
    f32 = mybir.dt.float32
    Exp = mybir.ActivationFunctionType.Exp
    Add = mybir.AluOpType.add
    nc = bacc.Bacc(
        "TRN2", target_bir_lowering=False, debug=False, num_devices=N_CORES
    )

    # Every DMA piece gets its own contiguous dram tensor: column-sliced
    # views of a [ROWS, W] tensor are 2-4 KiB chunks at 8 KiB stride in
    # HBM (row-buffer thrash, ~half rate); per-piece tensors make every
    # transfer a fully sequential HBM burst. The host packs/unpacks.
    i_b0 = [
        nc.dram_tensor(f"b0_{t}", [P, H], f16, kind="ExternalInput")
        for t in ("uh0", "uh1", "lh0", "lh1")
    ]
    i_b1 = [
        nc.dram_tensor(f"b1_{t}", [P, N], f16, kind="ExternalInput")
        for t in ("l", "u")
    ]
    i_b2 = [
        nc.dram_tensor(f"b2_{t}", [P, N], f16, kind="ExternalInput")
        for t in ("l", "u")
    ]
    i_b3 = nc.dram_tensor("b3_xu", [P, W], f16, kind="ExternalInput")
    o_b0 = [
        nc.dram_tensor(f"o0_{t}", [P, N], bf16, kind="ExternalOutput")
        for t in ("lo", "up")
    ]
    o_b1 = nc.dram_tensor("o1", [P, W], bf16, kind="ExternalOutput")
    o_b2 = nc.dram_tensor("o2", [P, W], bf16, kind="ExternalOutput")
    o_b3 = [
        nc.dram_tensor(f"o3_{t}", [P, H], bf16, kind="ExternalOutput")
        for t in ("loh0", "loh1", "uph0", "uph1")
    ]

    with tile.TileContext(nc) as tc:
        with (
            tc.tile_pool(name="io", bufs=3) as io,
            tc.tile_pool(name="eb", bufs=3) as eb,
            tc.tile_pool(name="ob", bufs=3) as ob,
            tc.tile_pool(name="stats", bufs=8) as st,
        ):
            # Phase 1: all input DMAs up front (io bufs cover all 4
            # blocks) so no output-DMA wait ever stalls an input issue.
            # Block 0 streams in as quarters, u first, so the first
            # lower-side DVE op can start as early as possible.
            xus = []
            for b in range(NBLK):
                xu = io.tile([P, W], f16, tag="xu")
                if b == 0:
                    nc.sync.dma_start(out=xu[:, N : N + H], in_=i_b0[0][:, :])
                    nc.sync.dma_start(out=xu[:, N + H :], in_=i_b0[1][:, :])
                    nc.sync.dma_start(out=xu[:, 0:H], in_=i_b0[2][:, :])
                    nc.sync.dma_start(out=xu[:, H:N], in_=i_b0[3][:, :])
                elif b in (1, 2):
                    # halves so each block's exp_l isn't starved
                    src_lu = i_b1 if b == 1 else i_b2
                    nc.sync.dma_start(out=xu[:, :N], in_=src_lu[0][:, :])
                    nc.sync.dma_start(out=xu[:, N:], in_=src_lu[1][:, :])
                else:
                    nc.sync.dma_start(out=xu, in_=i_b3[:, :])
                xus.append(xu)

            # Phase 2: per-block compute + store.
            for b in range(NBLK):
                rows = slice(b * P, (b + 1) * P)
                xu = xus[b]
                e = eb.tile([P, W], f32, tag="e")
                s = st.tile([P, 6], f32, tag="s")
                o = ob.tile([P, W], bf16, tag="o")

                if b == 0:
                    # cols: s0=S_l_h0, s1=S_l_h1, s2=S_u_h0, s3=S_u_h1,
                    #       s4=S_u, s5=S_l
                    nc.scalar.activation(
                        e[:, N : N + H], xu[:, N : N + H], Exp,
                        accum_out=s[:, 2:3],
                    )
                    nc.scalar.activation(
                        e[:, N + H :], xu[:, N + H :], Exp, accum_out=s[:, 3:4]
                    )
                    nc.scalar.activation(
                        e[:, 0:H], xu[:, 0:H], Exp, accum_out=s[:, 0:1]
                    )
                    nc.scalar.activation(
                        e[:, H:N], xu[:, H:N], Exp, accum_out=s[:, 1:2]
                    )
                    nc.vector.tensor_scalar(
                        s[:, 4:5], s[:, 2:3], s[:, 3:4], None, op0=Add
                    )
                    nc.vector._custom_dve(
                        op, out=o[:, 0:H], in0=e[:, 0:H], in1=e[:, N : N + H],
                        s0=s[:, 4:5], s1=_SEED_C, imm2=_NR_C,
                    )
                    nc.vector._custom_dve(
                        op, out=o[:, H:N], in0=e[:, H:N], in1=e[:, N + H :],
                        s0=s[:, 4:5], s1=_SEED_C, imm2=_NR_C,
                    )
                    nc.sync.dma_start(out=o_b0[0][:, :], in_=o[:, :N])
                    nc.vector.tensor_scalar(
                        s[:, 5:6], s[:, 0:1], s[:, 1:2], None, op0=Add
                    )
                    nc.vector._custom_dve(
                        op, out=o[:, N:], in0=e[:, N:], in1=e[:, :N],
                        s0=s[:, 5:6], s1=_SEED_C, imm2=_NR_C,
                    )
                    nc.sync.dma_start(out=o_b0[1][:, :], in_=o[:, N:])
                elif b < NBLK - 1:
                    # cols: s[:,0]=S_l, s[:,1]=S_u
                    nc.scalar.activation(
                        e[:, :N], xu[:, :N], Exp, accum_out=s[:, 0:1]
                    )
                    nc.scalar.activation(
                        e[:, N:], xu[:, N:], Exp, accum_out=s[:, 1:2]
                    )
                    # lower = e_l * recip1(e_l - e_u + S_u)
                    nc.vector._custom_dve(
                        op, out=o[:, :N], in0=e[:, :N], in1=e[:, N:],
                        s0=s[:, 1:2], s1=_SEED_C, imm2=_NR_C,
                    )
                    # upper = e_u * recip1(e_u - e_l + S_l)
                    nc.vector._custom_dve(
                        op, out=o[:, N:], in0=e[:, N:], in1=e[:, :N],
                        s0=s[:, 0:1], s1=_SEED_C, imm2=_NR_C,
                    )
                    nc.sync.dma_start(
                        out=(o_b1 if b == 1 else o_b2)[:, :], in_=o
                    )
                else:
                    # Last block: exp(u) first, then exp(l) in column
                    # halves; the lower-side DVE ops chase the halves,
                    # and upper (gated on full S_l) runs in halves with
                    # quarter stores so the post-ACT tail is short.
                    # cols: s[:,0]=S_l_h0, s[:,1]=S_l_h1, s[:,2]=S_u,
                    #       s[:,3]=S_l
                    nc.scalar.activation(
                        e[:, N:], xu[:, N:], Exp, accum_out=s[:, 2:3]
                    )
                    nc.scalar.activation(
                        e[:, 0:H], xu[:, 0:H], Exp, accum_out=s[:, 0:1]
                    )
                    nc.scalar.activation(
                        e[:, H:N], xu[:, H:N], Exp, accum_out=s[:, 1:2]
                    )
                    nc.vector._custom_dve(
                        op, out=o[:, 0:H], in0=e[:, 0:H], in1=e[:, N : N + H],
                        s0=s[:, 2:3], s1=_SEED_C, imm2=_NR_C,
                    )
                    nc.sync.dma_start(out=o_b3[0][:, :], in_=o[:, 0:H])
                    nc.vector._custom_dve(
                        op, out=o[:, H:N], in0=e[:, H:N], in1=e[:, N + H :],
                        s0=s[:, 2:3], s1=_SEED_C, imm2=_NR_C,
                    )
                    nc.sync.dma_start(out=o_b3[1][:, :], in_=o[:, H:N])
                    nc.vector.tensor_scalar(
                        s[:, 3:4], s[:, 0:1], s[:, 1:2], None, op0=Add
                    )
                    nc.vector._custom_dve(
                        op, out=o[:, N : N + H], in0=e[:, N : N + H],
                        in1=e[:, 0:H], s0=s[:, 3:4], s1=_SEED_C, imm2=_NR_C,
                    )
                    nc.sync.dma_start(
                        out=o_b3[2][:, :], in_=o[:, N : N + H]
                    )
                    nc.vector._custom_dve(
                        op, out=o[:, N + H :], in0=e[:, N + H :],
                        in1=e[:, H:N], s0=s[:, 3:4], s1=_SEED_C, imm2=_NR_C,
                    )
                    nc.sync.dma_start(
                        out=o_b3[3][:, :], in_=o[:, N + H :]
                    )

    nc.compile()
    return nc


def _get_nc():
    if "nc" not in _cache:
        _cache["nc"] = _build()
    return _cache["nc"]


def kernel(l: np.ndarray, u: np.ndarray):
    from concourse import bass_utils

    assert l.shape == (B, N) and u.shape == (B, N)
    lh = np.ascontiguousarray(l, dtype=np.float16)
    uh = np.ascontiguousarray(u, dtype=np.float16)

    def core_inputs(i):
        r = i * ROWS
        cp = np.ascontiguousarray
        return {
            "b0_uh0": cp(uh[r : r + P, 0:H]),
            "b0_uh1": cp(uh[r : r + P, H:N]),
            "b0_lh0": cp(lh[r : r + P, 0:H]),
            "b0_lh1": cp(lh[r : r + P, H:N]),
            "b1_l": cp(lh[r + P : r + 2 * P]),
            "b1_u": cp(uh[r + P : r + 2 * P]),
            "b2_l": cp(lh[r + 2 * P : r + 3 * P]),
            "b2_u": cp(uh[r + 2 * P : r + 3 * P]),
            "b3_xu": np.concatenate(
                [lh[r + 3 * P : r + 4 * P], uh[r + 3 * P : r + 4 * P]], axis=1
            ),
        }

    nc = _get_nc()
    in_maps = [core_inputs(i) for i in range(N_CORES)]
    trace = bool(int(os.environ.get("KERNEL_TRACE", "0")))
    res = bass_utils.run_bass_kernel_spmd(
        nc,
        in_maps,
        core_ids=list(range(N_CORES)),
        trace=trace,
        trace_cores=[0] if trace else None,
    )
    _cache["last_run"] = res
    lower = np.empty((B, N), dtype=np.float32)
    upper = np.empty((B, N), dtype=np.float32)
    for i, r_ in enumerate(res.results):
        r = i * ROWS
        g = lambda name: np.asarray(r_[name]).astype(np.float32)
        lower[r : r + P] = g("o0_lo")
        upper[r : r + P] = g("o0_up")
        o1 = g("o1")
        lower[r + P : r + 2 * P] = o1[:, :N]
        upper[r + P : r + 2 * P] = o1[:, N:]
        o2 = g("o2")
        lower[r + 2 * P : r + 3 * P] = o2[:, :N]
        upper[r + 2 * P : r + 3 * P] = o2[:, N:]
        lower[r + 3 * P : r + 4 * P, 0:H] = g("o3_loh0")
        lower[r + 3 * P : r + 4 * P, H:N] = g("o3_loh1")
        upper[r + 3 * P : r + 4 * P, 0:H] = g("o3_uph0")
        upper[r + 3 * P : r + 4 * P, H:N] = g("o3_uph1")
    return lower, upper
